# revision 21
# baseline (speedup 1.0000x reference)
"""Trainium2 distributed Bass kernel for nn_ActorNetAblation (GNN message passing).

Sharding: nodes split into 8 ranges of 6250 (padded 6272=49*128 per core);
edges sharded by dst range so segment-sum is core-local; per-iteration
AllGather (bf16 node table) feeds per-edge gathers of out[src].

v2 restructure vs baseline:
  - Per-edge gathers split across GpSimd Q7 core pairs: indirect_dma_start
    (cores 0/1, exact 64B rows, no select) for most batches, dma_gather on
    queues 1-3 (cores 2-7, 256B packs + 4-way select) for the rest.
  - seT one-hot matrices are iteration-invariant: built once into SBUF.
  - invdeg folded into the wedge build (ACT per-partition scale).
  - nn2 bias folded into the wedge matmul via a ones-row (no bias matmul).
  - Node phase (GRU) interleaved into the edge loop per 4-window chunk.

Edge math per 128-edge tile:
  tmp[e,(o,i)] = Wedge[e,(o,i)] * out[src_e, i]        (Wedge pre-scaled
  psum_win    += SeT.T @ tmp[:, :, g*16:+16] for g in 0,1   by invdeg[dst])
  agg_win      = reduce_i16(psum_win)  per closed window pair
SPMD: all 8 cores run ONE graph; per-core differences flow through inputs.
"""

import numpy as np

N, E, T, D = 50000, 160000, 8192, 32
C = 8
NS = 6250
NSP = 6272
W = 49
ITERS = 6
TCORE = T // C
WPB = 4            # windows per batch (= per node chunk)
# batches via dma_gather (queues 1-3; cores 0/1 kept free for indirect)
GATHER_BATCHES = {0: 1, 2: 2, 4: 3, 6: 1, 8: 2, 10: 3, 12: 1}

_cache = {}


def _bf(a):
    import ml_dtypes
    return np.asarray(a).astype(ml_dtypes.bfloat16)


def _host_prep(inputs):
    x = np.asarray(inputs["x"], np.float32)
    ei = np.asarray(inputs["edge_index"]).astype(np.int64)
    nonring = np.asarray(inputs["nonring"]).astype(np.int64)
    ea = np.asarray(inputs["edge_attr"], np.float32)

    src, dst = ei[0], ei[1]
    deg = np.maximum(
        np.bincount(dst, minlength=N).astype(np.float32), 1.0)
    invdeg_all = 1.0 / deg

    def table_row(g):
        return (g // NS) * NSP + (g % NS)

    shard_of = dst // NS
    maxcnt = 0
    percore = []
    for c in range(C):
        m = shard_of == c
        s_c, d_c, ea_c = src[m], dst[m], ea[m]
        dloc = d_c - c * NS
        win = dloc // 128
        order = np.argsort(win, kind="stable")
        s_c, ea_c, dloc, win = (a[order] for a in (s_c, ea_c, dloc, win))
        cnts = np.bincount(win, minlength=W)
        maxcnt = max(maxcnt, int(cnts.max()))
        percore.append((s_c, ea_c, dloc, cnts))

    TPW = max(4, -(-maxcnt // 128))
    TILES = W * TPW
    EP = TILES * 128

    w = {k: np.asarray(inputs[k], np.float32) for k in (
        "lin0_w", "lin0_b", "nn1_w", "nn1_b", "nn2_w", "nn2_b",
        "conv_root", "conv_b", "gru_w_ih", "gru_w_hh", "gru_b_ih",
        "gru_b_hh", "lstm_w_ih", "lstm_w_hh", "lstm_b_ih", "lstm_b_hh",
        "lin1_w", "lin1_b", "lin2_w", "lin2_b")}

    perm = (np.arange(D * D).reshape(D, D).T).reshape(-1)
    nn1_w9 = np.zeros((8, 33), np.float32)
    nn1_w9[:7, :32] = w["nn1_w"]
    nn1_w9[7, :32] = w["nn1_b"]
    nn1_w9[7, 32] = 1.0
    nn2_wPb = np.zeros((33, D * D), np.float32)
    nn2_wPb[:32] = w["nn2_w"][:, perm]
    nn2_wPb[32] = w["nn2_b"][perm]
    weights = {
        "nn1_w9": nn1_w9,
        "nn2_wPb": nn2_wPb,
        "lin0_w4": np.concatenate([w["lin0_w"], w["lin0_b"][None]], 0),
        "conv_root": w["conv_root"],
        "wih_r": w["gru_w_ih"][:, :D], "wih_z": w["gru_w_ih"][:, D:2 * D],
        "wih_n": w["gru_w_ih"][:, 2 * D:],
        "whh_r": w["gru_w_hh"][:, :D], "whh_z": w["gru_w_hh"][:, D:2 * D],
        "whh_n": w["gru_w_hh"][:, 2 * D:],
        "lin1_wA": w["lin1_w"][:128], "lin2_w": w["lin2_w"],
    }
    for gi, g in enumerate("ifgo"):
        sl = slice(gi * D, (gi + 1) * D)
        weights[f"lstmA_{g}"] = w["lstm_w_ih"][:D, sl]
        weights[f"lstmB_{g}"] = w["lstm_w_ih"][D:, sl]
        weights[f"lstmH_{g}"] = w["lstm_w_hh"][:, sl]

    grub = w["gru_b_ih"] + w["gru_b_hh"]
    lstmb = w["lstm_b_ih"] + w["lstm_b_hh"]
    col_arrays = {
        "conv_b": w["conv_b"], "b_r": grub[:D], "b_z": grub[D:2 * D],
        "b_ihn": w["gru_b_ih"][2 * D:],
        "lin1_b": w["lin1_b"], "lin2_b": w["lin2_b"],
    }
    for gi, g in enumerate("ifgo"):
        col_arrays[f"lstmb_{g}"] = lstmb[gi * D:(gi + 1) * D]
    colnames = sorted(col_arrays)
    cols = np.zeros((128, len(colnames)), np.float32)
    for i, n in enumerate(colnames):
        a = col_arrays[n]
        cols[:len(a), i] = a
    # row-shaped constants: [b_hhn, sbar]
    rows = np.zeros((1, 2 * D), np.float32)
    rows[0, :D] = w["gru_b_hh"][2 * D:]
    rows[0, D:] = w["lin1_w"][128:].sum(0)

    shared = {k: _bf(v) for k, v in weights.items()}
    shared["cols"] = cols
    shared["rows"] = _bf(rows)

    in_maps = []
    for c in range(C):
        s_c, ea_c, dloc, cnts = percore[c]
        eaT8 = np.zeros((8, EP), np.float32)
        srcrow = np.zeros((EP,), np.int32)
        dstrel = np.full((EP,), -1.0, np.float32)
        invdeg = np.ones((EP,), np.float32)
        ptr = 0
        for wi in range(W):
            n = int(cnts[wi])
            base = wi * TPW * 128
            sl = slice(ptr, ptr + n)
            eaT8[:7, base:base + n] = ea_c[sl].T
            eaT8[7, base:base + n] = 1.0
            srcrow[base:base + n] = table_row(s_c[sl]).astype(np.int32)
            dstrel[base:base + n] = (dloc[sl] - wi * 128).astype(np.float32)
            invdeg[base:base + n] = invdeg_all[dloc[sl] + c * NS]
            ptr += n

        def lane(a):
            return np.ascontiguousarray(a.reshape(TILES, 128).T)

        srcrow_l = lane(srcrow)                       # [128, TILES]
        rem = (srcrow_l % 4).astype(np.int64)
        ent = (srcrow_l // 4).astype(np.int16)        # [128, TILES]
        invdeg_l = lane(invdeg)
        mask4 = np.zeros((128, TILES, 4), np.float32)
        for j in range(4):
            mask4[:, :, j] = (rem == j)
        mask4 = mask4.reshape(128, TILES * 4)
        # wrapped idx layout for dma_gather: [16, nidx/16] replicated x8
        gidx = np.zeros((128, TILES * 8), np.int16)
        p_ = np.arange(128)
        for t in range(TILES):
            gidx[p_ % 16, t * 8 + p_ // 16] = ent[:, t]
        gidx = np.tile(gidx[:16], (8, 1))

        x4T = np.zeros((4, NSP), np.float32)
        x4T[:3, :NS] = x[c * NS:(c + 1) * NS].T
        x4T[3, :NS] = 1.0
        padmask = np.zeros((128, W), np.float32)
        idx = np.arange(NSP).reshape(W, 128).T
        padmask[idx < NS] = 1.0
        nrf = nonring.reshape(-1)
        cc_, u_ = np.meshgrid(np.arange(128), np.arange(32), indexing="ij")
        g4idx = table_row(nrf[cc_ * 256 + 32 * c + u_]).astype(np.int32)
        selA = np.zeros((32, TCORE), np.float32)
        selB = np.zeros((32, TCORE), np.float32)
        for b in range(8):
            mw = 8 * c + b
            (selA if mw < 32 else selB)[mw % 32, 128 * b:128 * (b + 1)] = 1.0
        m = {
            "eaT8": _bf(eaT8), "gidx": gidx, "srcrow": srcrow_l,
            "dstrel": _bf(lane(dstrel)), "mask4": _bf(mask4),
            "invdegl": invdeg_l,
            "x4T": _bf(x4T), "padmask": padmask, "g4idx": g4idx,
            "selA": _bf(selA), "selB": _bf(selB),
        }
        m.update({k: v.copy() for k, v in shared.items()})
        in_maps.append(m)
    return in_maps, weights, colnames, TPW, TILES


def _build_graph(weights, colnames, TPW, TILES):
    import os
    PHASE = int(os.environ.get("KDBG_PHASE", "99"))
    import concourse.bacc as bacc
    import concourse.bass as bass
    import concourse.mybir as mybir
    import concourse.tile as tile
    from concourse.masks import make_identity

    f32 = mybir.dt.float32
    bf16 = mybir.dt.bfloat16
    i32 = mybir.dt.int32
    i16 = mybir.dt.int16
    AF = mybir.ActivationFunctionType
    OP = mybir.AluOpType
    EP = TILES * 128
    RG = [list(range(C))]
    NCOL = len(colnames)

    nc = bacc.Bacc("TRN2", target_bir_lowering=False, debug=False,
                   num_devices=C, num_swdge_queues=4)

    din = {}
    def dI(name, shape, dt):
        din[name] = nc.dram_tensor(name, shape, dt, kind="ExternalInput")
        return din[name]

    dI("eaT8", [8, EP], bf16)
    dI("gidx", [128, TILES * 8], i16)
    dI("srcrow", [128, TILES], i32)
    dI("dstrel", [128, TILES], bf16)
    dI("mask4", [128, TILES * 4], bf16)
    dI("invdegl", [128, TILES], f32)
    dI("x4T", [4, NSP], bf16)
    dI("padmask", [128, W], f32)
    dI("g4idx", [128, 32], i32)
    dI("selA", [32, TCORE], bf16)
    dI("selB", [32, TCORE], bf16)
    dI("cols", [128, NCOL], f32)
    dI("rows", [1, 2 * D], bf16)
    for k, v in weights.items():
        dI(k, list(v.shape), bf16)
    out_d = nc.dram_tensor("out", [TCORE, 6], f32, kind="ExternalOutput")

    with tile.TileContext(nc) as tc:
        with (
            tc.tile_pool(name="tablep", bufs=1, space="DRAM") as table_pool,
            tc.tile_pool(name="aginp", bufs=1, space="DRAM") as agin_pool,
            tc.tile_pool(name="whbmp", bufs=1, space="DRAM") as whbm_pool,
            tc.tile_pool(name="arinp", bufs=1, space="DRAM") as arin_pool,
            tc.tile_pool(name="aroutp", bufs=1, space="DRAM") as arout_pool,
            tc.tile_pool(name="pp", bufs=1) as pp,
            tc.tile_pool(name="mtp", bufs=1) as mtp,
            tc.tile_pool(name="bld", bufs=2) as bld,
            tc.tile_pool(name="wsbp", bufs=3) as wsbp,
            tc.tile_pool(name="wedge", bufs=2) as wedge_pool,
            tc.tile_pool(name="gath", bufs=5) as gath,
            tc.tile_pool(name="gtp", bufs=3) as gtp,
            tc.tile_pool(name="tmpp", bufs=2) as tmpp,
            tc.tile_pool(name="nsb", bufs=2) as nsb,
            tc.tile_pool(name="ps", bufs=2, space="PSUM") as ps,
        ):
            tables = [table_pool.tile([C * NSP, D], bf16,
                                      addr_space="Shared", tag=f"tab{k}",
                                      name=f"tab{k}")
                      for k in range(ITERS + 1)]
            agins = [agin_pool.tile([NSP, D], bf16, tag=f"agin{k}",
                                    name=f"agin{k}")
                     for k in range(ITERS + 1)]
            whbm = whbm_pool.tile([EP, 1024], bf16)
            ar_ins = [arin_pool.tile([D + 1, 1], f32, tag=f"ari{k}",
                                     name=f"ari{k}")
                      for k in range(ITERS)]
            ar_outs = [arout_pool.tile([D + 1, 1], f32, addr_space="Shared",
                                       tag=f"aro{k}", name=f"aro{k}")
                       for k in range(ITERS)]

            # ---- static loads ------------------------------------------
            def load(name, dt=bf16):
                t = pp.tile([s for s in din[name].shape], dt,
                            tag=f"ld_{name}")
                nc.sync.dma_start(t[:], din[name].ap())
                return t

            gidx_s = load("gidx", i16)
            srcrow_s = load("srcrow", i32)
            mask4_s = load("mask4")
            dstrel_s = load("dstrel")
            invdeg_s = load("invdegl", f32)
            padmask_s = load("padmask", f32)
            g4idx_s = load("g4idx", i32)
            selA_s = load("selA")
            selB_s = load("selB")
            cols_s = load("cols", f32)
            rows_s = load("rows")
            wb = {k: load(k) for k in weights}

            def col(name, n=D):
                i = colnames.index(name)
                return cols_s[:n, i:i + 1]

            bhhn_row = rows_s[:, :D]
            sbar_row = rows_s[:, D:]

            iota_i = pp.tile([128, 128], i32)
            nc.gpsimd.iota(iota_i[:], pattern=[[1, 128]], base=0,
                           channel_multiplier=0)
            iota_b = pp.tile([128, 128], bf16)
            nc.vector.tensor_copy(out=iota_b[:], in_=iota_i[:])

            ident = pp.tile([128, 128], f32)
            make_identity(nc, ident[:])
            identb = pp.tile([128, 128], bf16)
            nc.vector.tensor_copy(out=identb[:], in_=ident[:])

            ones_r128 = pp.tile([1, 128], bf16)
            nc.vector.memset(ones_r128[:], 1.0)
            ones_r512 = pp.tile([1, 512], bf16)
            nc.vector.memset(ones_r512[:], 1.0)
            ones_c128 = pp.tile([128, 1], bf16)
            nc.vector.memset(ones_c128[:], 1.0)

            outT = pp.tile([D, NSP], bf16)
            out_sb = pp.tile([128, W * D], bf16)
            agg_sb = pp.tile([128, W * D], f32)

            # ---- seT prebuild (iteration-invariant one-hots) -----------
            seT_all = pp.tile([128, TILES * 128], bf16)
            for t4 in range(TILES // 4):
                dv = dstrel_s[:, t4 * 4:t4 * 4 + 4].unsqueeze(2)
                nc.vector.tensor_tensor(
                    out=seT_all[:, t4 * 512:(t4 + 1) * 512].rearrange(
                        "p (k n) -> p k n", n=128),
                    in0=dv.to_broadcast([128, 4, 128]),
                    in1=iota_b[:].unsqueeze(1).to_broadcast([128, 4, 128]),
                    op=OP.is_equal)

            NCH = [(i * 512, min(512, NSP - i * 512))
                   for i in range((NSP + 511) // 512)]

            def table_chunk(k, wlist):
                """Transpose outT windows -> out_sb, DMA slice to agins[k]."""
                for wi in wlist:
                    tp = ps.tile([128, D], bf16, tag="small")
                    nc.tensor.transpose(
                        tp[:], outT[:, wi * 128:(wi + 1) * 128],
                        identb[:D, :D])
                    nc.scalar.copy(
                        out=out_sb[:, wi * D:(wi + 1) * D], in_=tp[:])
                w0, w1 = wlist[0], wlist[-1] + 1
                nc.sync.dma_start(
                    agins[k][w0 * 128:w1 * 128, :].rearrange(
                        "(w p) f -> p w f", p=128),
                    out_sb[:, w0 * D:w1 * D].rearrange(
                        "p (w f) -> p w f", f=D))

            def allgather(k):
                nc.gpsimd.collective_compute(
                    "AllGather", mybir.AluOpType.bypass,
                    replica_groups=RG,
                    ins=[agins[k][:].opt()], outs=[tables[k][:].opt()])

            # ---- init: lin0 --------------------------------------------
            for ci, (c0, cn) in enumerate(NCH):
                x4c = bld.tile([4, 512], bf16, tag="x4c")
                nc.sync.dma_start(x4c[:, :cn], din["x4T"].ap()[:, c0:c0 + cn])
                ip = ps.tile([D, 512], f32, tag="med")
                nc.tensor.matmul(ip[:, :cn], lhsT=wb["lin0_w4"][:],
                                 rhs=x4c[:, :cn], start=True,
                                 stop=True)
                nc.scalar.activation(outT[:, c0:c0 + cn], ip[:, :cn],
                                     AF.Relu)
            table_chunk(0, list(range(W)))
            allgather(0)

            # ---- wedge build -------------------------------------------
            BBT = 14
            for t in range(TILES if PHASE >= 2 else 0):
                if t % BBT == 0:
                    nbt = min(BBT, TILES - t)
                    ea_c = bld.tile([8, BBT * 128], bf16, tag="ea")
                    nc.sync.dma_start(
                        ea_c[:, :nbt * 128],
                        din["eaT8"].ap()[:, t * 128:(t + nbt) * 128])
                if t % 2 == 0:
                    nb2 = min(2, TILES - t)
                    rps = ps.tile([33, 256], f32, tag="small")
                    nc.tensor.matmul(rps[:, :nb2 * 128],
                                     lhsT=wb["nn1_w9"][:],
                                     rhs=ea_c[:, (t % BBT) * 128:
                                              (t % BBT + nb2) * 128],
                                     start=True, stop=True)
                    r33b = bld.tile([33, 256], bf16, tag="r33")
                    nc.scalar.activation(r33b[:, :nb2 * 128], rps[:, :nb2 * 128],
                                         AF.Relu)
                r33 = r33b[:, (t % 2) * 128:(t % 2 + 1) * 128]
                wsb = wsbp.tile([128, 1024], bf16, tag="wsb")
                for j in range(2):
                    wps = ps.tile([128, 512], f32, tag="med")
                    nc.tensor.matmul(
                        wps[:], lhsT=r33,
                        rhs=wb["nn2_wPb"][:, j * 512:(j + 1) * 512],
                        start=True, stop=True)
                    if (t + j) % 2 == 0:
                        nc.scalar.activation(
                            wsb[:, j * 512:(j + 1) * 512], wps[:],
                            AF.Identity, scale=invdeg_s[:, t:t + 1])
                    else:
                        nc.vector.tensor_scalar(
                            out=wsb[:, j * 512:(j + 1) * 512], in0=wps[:],
                            scalar1=invdeg_s[:, t:t + 1], scalar2=None,
                            op0=OP.mult)
                nc.sync.dma_start(whbm[t * 128:(t + 1) * 128, :], wsb[:])

            # ---- message passing ---------------------------------------
            mT = pp.tile([D, NSP], bf16)
            NBATCH = -(-W // WPB)
            for it in range(min(ITERS, max(0, PHASE - 2))):
              for bb in range(NBATCH + 1):
                if bb < NBATCH:
                    b = bb
                    wl = list(range(b * WPB, min((b + 1) * WPB, W)))
                    nt = len(wl) * TPW            # tiles in this batch
                    t0 = b * WPB * TPW
                    # gather batch: oss_b [128, nt*D] = out[src] rows
                    oss_b = gath.tile([128, WPB * TPW * D], bf16,
                                      tag="oss_b")
                    if b in GATHER_BATCHES:
                        gt = gtp.tile([128, WPB * TPW * 128], bf16,
                                      tag="gt")
                        # split into 4 quarter-gathers on queues 0-3: the
                        # Q7 core pairs run them concurrently (~4x faster)
                        nq = -(-nt // TPW) // 1   # windows in batch
                        for qq, tq in enumerate(range(0, nt, TPW)):
                            nqt = min(TPW, nt - tq)
                            nc.gpsimd.dma_gather(
                                out_ap=gt[:, tq * 128:(tq + nqt) * 128
                                          ].rearrange(
                                    "p (s f) -> p s f", f=128),
                                in_ap=tables[it][:].rearrange(
                                    "(a b) f -> a (b f)", b=4),
                                idxs_ap=gidx_s[:, (t0 + tq) * 8:
                                               (t0 + tq + nqt) * 8],
                                num_idxs=nqt * 128, num_idxs_reg=nqt * 128,
                                elem_size=128, single_packet=False,
                                queue_num=1 + (qq + b) % 3)
                        gvv = gt[:].rearrange("p (s j i) -> p s j i", j=4,
                                              i=D)
                        mkv = mask4_s[:, 4 * t0:4 * (t0 + nt)]
                        mkv = mkv.rearrange("p (s j) -> p s j", j=4)
                        ob3 = oss_b[:, :nt * D].rearrange(
                            "p (s i) -> p s i", i=D)
                        acc = gtp.tile([128, WPB * TPW * D], bf16,
                                       tag="acc")
                        ac3 = acc[:, :nt * D].rearrange(
                            "p (s i) -> p s i", i=D)
                        nc.vector.tensor_tensor(
                            out=ob3, in0=gvv[:, :nt, 0, :],
                            in1=mkv[:, :nt, 0:1].to_broadcast([128, nt, D]),
                            op=OP.mult)
                        for j in range(1, 4):
                            nc.vector.tensor_tensor(
                                out=ac3, in0=gvv[:, :nt, j, :],
                                in1=mkv[:, :nt, j:j + 1].to_broadcast(
                                    [128, nt, D]),
                                op=OP.mult)
                            nc.vector.tensor_tensor(
                                out=ob3, in0=ob3, in1=ac3, op=OP.add)
                    else:
                        for k in range(nt):
                            nc.gpsimd.indirect_dma_start(
                                out=oss_b[:, k * D:(k + 1) * D],
                                out_offset=None,
                                in_=tables[it][:],
                                in_offset=bass.IndirectOffsetOnAxis(
                                    ap=srcrow_s[:, t0 + k:t0 + k + 1],
                                    axis=0))

                    for wi in wl:                 # one window = TPW tiles
                        t = wi * TPW
                        tl = t - t0               # tile offset in batch
                        wtw = wedge_pool.tile([128, TPW * 1024], bf16,
                                              tag="wtw")
                        eng = nc.sync if wi % 2 == 0 else nc.scalar
                        eng.dma_start(
                            wtw[:].rearrange("p (k f) -> p k f", f=1024),
                            whbm[t * 128:(t + TPW) * 128, :].rearrange(
                                "(k p) f -> p k f", p=128))
                        tmpb = tmpp.tile([128, TPW * 1024], bf16,
                                         tag="tmpb")
                        nc.vector.tensor_tensor(
                            out=tmpb[:].rearrange(
                                "p (s o i) -> p s o i", s=TPW, i=D),
                            in0=wtw[:].rearrange("p (s o i) -> p s o i",
                                                 s=TPW, i=D),
                            in1=oss_b[:, tl * D:(tl + TPW) * D].rearrange(
                                "p (s i) -> p s i", i=D).unsqueeze(
                                2).to_broadcast([128, TPW, D, D]),
                            op=OP.mult)
                        if wi % 2 == 0:
                            aggw = ps.tile([128, 1024], f32, tag="big")
                        half = (wi % 2) * 512
                        for ti in range(TPW):
                            tv = tmpb[:, ti * 1024:(ti + 1) * 1024
                                      ].rearrange("p (o i) -> p o i", i=D)
                            seT = seT_all[:, (t + ti) * 128:
                                          (t + ti + 1) * 128]
                            for g in range(2):
                                nc.tensor.matmul(
                                    aggw[:, half:half + 512], lhsT=seT,
                                    rhs=tv[:, :, g * 16:(g + 1) * 16],
                                    start=(ti == 0 and g == 0),
                                    stop=(ti == TPW - 1 and g == 1))
                        if wi % 2 == 1:
                            nc.vector.tensor_reduce(
                                out=agg_sb[:, (wi - 1) * D:(wi + 1) * D],
                                in_=aggw[:].rearrange(
                                    "p (w o i) -> p w o i", w=2, i=16),
                                axis=mybir.AxisListType.X, op=OP.add)
                        elif wi == W - 1:
                            nc.vector.tensor_reduce(
                                out=agg_sb[:, wi * D:(wi + 1) * D],
                                in_=aggw[:, :512].rearrange(
                                    "p (o i) -> p o i", i=16),
                                axis=mybir.AxisListType.X, op=OP.add)

                # ---- node chunk bb-1 (one batch behind) ------------
                if bb > 0:
                    nwl = list(range((bb - 1) * WPB, min(bb * WPB, W)))
                    c0 = nwl[0] * 128
                    cn = len(nwl) * 128
                    for wi in nwl:
                        mp = ps.tile([D, 128], f32, tag="small")
                        nc.tensor.transpose(
                            mp[:], agg_sb[:, wi * D:(wi + 1) * D],
                            ident[:, :128])
                        nc.tensor.matmul(mp[:], lhsT=wb["conv_root"][:],
                                         rhs=outT[:, wi * 128:(wi + 1) * 128],
                                         start=False, stop=True,
                                         skip_group_check=True)
                        nc.scalar.activation(mT[:, wi * 128:(wi + 1) * 128],
                                             mp[:], AF.Relu,
                                             bias=col("conv_b"))
                    rp = ps.tile([D, 512], f32, tag="med")
                    zp = ps.tile([D, 512], f32, tag="med")
                    for ps_, wi_, wh_ in ((rp, "wih_r", "whh_r"),
                                          (zp, "wih_z", "whh_z")):
                        nc.tensor.matmul(ps_[:, :cn], lhsT=wb[wi_][:],
                                         rhs=mT[:, c0:c0 + cn], start=True,
                                         stop=False)
                        nc.tensor.matmul(ps_[:, :cn], lhsT=wb[wh_][:],
                                         rhs=outT[:, c0:c0 + cn],
                                         start=False, stop=True)
                    r_sb = nsb.tile([D, 512], bf16, tag="r_sb")
                    z_sb = nsb.tile([D, 512], bf16, tag="z_sb")
                    nc.scalar.activation(r_sb[:, :cn], rp[:, :cn],
                                         AF.Sigmoid, bias=col("b_r"))
                    nc.scalar.activation(z_sb[:, :cn], zp[:, :cn],
                                         AF.Sigmoid, bias=col("b_z"))
                    xnp = ps.tile([D, 512], f32, tag="med")
                    hnp = ps.tile([D, 512], f32, tag="med")
                    nc.tensor.matmul(xnp[:, :cn], lhsT=wb["wih_n"][:],
                                     rhs=mT[:, c0:c0 + cn], start=True,
                                     stop=True)
                    nc.tensor.matmul(hnp[:, :cn], lhsT=wb["whh_n"][:],
                                     rhs=outT[:, c0:c0 + cn], start=True,
                                     stop=False)
                    nc.tensor.matmul(hnp[:, :cn], lhsT=bhhn_row[:],
                                     rhs=ones_r512[:, :cn], start=False,
                                     stop=True)
                    hn_sb = nsb.tile([D, 512], bf16, tag="hn_sb")
                    nc.scalar.copy(out=hn_sb[:, :cn], in_=hnp[:, :cn])
                    xn_sb = nsb.tile([D, 512], bf16, tag="xn_sb")
                    nc.scalar.copy(out=xn_sb[:, :cn], in_=xnp[:, :cn])
                    t1 = nsb.tile([D, 512], bf16, tag="t1")
                    nc.vector.tensor_tensor(out=t1[:, :cn],
                                            in0=r_sb[:, :cn],
                                            in1=hn_sb[:, :cn], op=OP.mult)
                    t2 = nsb.tile([D, 512], bf16, tag="t2")
                    nc.vector.tensor_tensor(out=t2[:, :cn], in0=t1[:, :cn],
                                            in1=xn_sb[:, :cn], op=OP.add)
                    n_sb = nsb.tile([D, 512], bf16, tag="n_sb")
                    nc.scalar.activation(n_sb[:, :cn], t2[:, :cn], AF.Tanh,
                                         bias=col("b_ihn"))
                    u = nsb.tile([D, 512], bf16, tag="u")
                    nc.vector.tensor_tensor(out=u[:, :cn],
                                            in0=outT[:, c0:c0 + cn],
                                            in1=n_sb[:, :cn],
                                            op=OP.subtract)
                    v = nsb.tile([D, 512], bf16, tag="v")
                    nc.vector.tensor_tensor(out=v[:, :cn], in0=z_sb[:, :cn],
                                            in1=u[:, :cn], op=OP.mult)
                    nc.vector.tensor_tensor(out=outT[:, c0:c0 + cn],
                                            in0=n_sb[:, :cn],
                                            in1=v[:, :cn],
                                            op=OP.add)
                    table_chunk(it + 1, nwl)
              allgather(it + 1)

            # ---- Set2Set -----------------------------------------------
            qs1 = pp.tile([D, 1], bf16)
            qs2 = pp.tile([D, 1], bf16)
            hl = pp.tile([D, 1], bf16)
            cl = pp.tile([D, 1], f32)
            for t_ in (qs1, qs2, hl, cl):
                nc.vector.memset(t_[:], 0.0)
            for s in range(ITERS if PHASE >= 9 else 0):
                gates = {}
                for g in "ifgo":
                    gp = ps.tile([D, 1], f32, tag="small")
                    nc.tensor.matmul(gp[:], lhsT=wb[f"lstmA_{g}"][:],
                                     rhs=qs1[:], start=True, stop=False)
                    nc.tensor.matmul(gp[:], lhsT=wb[f"lstmB_{g}"][:],
                                     rhs=qs2[:], start=False, stop=False)
                    nc.tensor.matmul(gp[:], lhsT=wb[f"lstmH_{g}"][:],
                                     rhs=hl[:], start=False, stop=True)
                    fn = AF.Tanh if g == "g" else AF.Sigmoid
                    gt = nsb.tile([D, 1], f32, tag=f"g_{g}")
                    nc.scalar.activation(gt[:], gp[:], fn,
                                         bias=col(f"lstmb_{g}"))
                    gates[g] = gt
                t1 = nsb.tile([D, 1], f32, tag="s1")
                nc.vector.tensor_tensor(out=t1[:], in0=gates["f"][:],
                                        in1=cl[:], op=OP.mult)
                t2 = nsb.tile([D, 1], f32, tag="s2")
                nc.vector.tensor_tensor(out=t2[:], in0=gates["i"][:],
                                        in1=gates["g"][:], op=OP.mult)
                nc.vector.tensor_tensor(out=cl[:], in0=t1[:], in1=t2[:],
                                        op=OP.add)
                tc_ = nsb.tile([D, 1], f32, tag="s3")
                nc.scalar.activation(tc_[:], cl[:], AF.Tanh)
                nc.vector.tensor_tensor(out=hl[:], in0=gates["o"][:],
                                        in1=tc_[:], op=OP.mult)
                # q as a row
                qrp = ps.tile([1, D], bf16, tag="small")
                nc.tensor.transpose(qrp[:], hl[:], identb[:D, :D])
                qrow = nsb.tile([1, D], bf16, tag="qrow")
                nc.vector.tensor_copy(out=qrow[:], in_=qrp[:])
                # q_rep = ones128 (x) q
                qrep_p = ps.tile([128, D], f32, tag="small")
                nc.tensor.matmul(qrep_p[:], lhsT=ones_r128[:], rhs=qrow[:],
                                 start=True, stop=True)
                qrep = nsb.tile([128, D], bf16, tag="qrep")
                nc.vector.tensor_copy(out=qrep[:], in_=qrep_p[:])
                tl = mtp.tile([128, W * D], bf16, tag="tl")
                nc.vector.tensor_tensor(
                    out=tl[:].rearrange("p (w f) -> p w f", f=D),
                    in0=out_sb[:].rearrange("p (w f) -> p w f", f=D),
                    in1=qrep[:].unsqueeze(1).to_broadcast([128, W, D]),
                    op=OP.mult)
                logit = nsb.tile([128, W], f32, tag="logit")
                nc.vector.tensor_reduce(
                    out=logit[:],
                    in_=tl[:].rearrange("p (w f) -> p w f", f=D),
                    axis=mybir.AxisListType.X, op=OP.add)
                ex = nsb.tile([128, W], f32, tag="ex")
                nc.scalar.activation(ex[:], logit[:], AF.Exp)
                exm = nsb.tile([128, W], f32, tag="exm")
                nc.vector.tensor_tensor(out=exm[:], in0=ex[:],
                                        in1=padmask_s[:], op=OP.mult)
                exb = nsb.tile([128, W], bf16, tag="exb")
                nc.vector.tensor_copy(out=exb[:], in_=exm[:])
                # packed per-core partials: [:, :D] = sum_w out*e, [:, D] = sum_w e
                packed = nsb.tile([128, D + 1], f32, tag="packed")
                tr = mtp.tile([128, W * D], bf16, tag="tr")
                nc.vector.tensor_tensor(
                    out=tr[:].rearrange("p (w f) -> p w f", f=D),
                    in0=out_sb[:].rearrange("p (w f) -> p w f", f=D),
                    in1=exb[:].unsqueeze(2).to_broadcast([128, W, D]),
                    op=OP.mult)
                nc.vector.tensor_reduce(
                    out=packed[:, :D],
                    in_=tr[:].rearrange("p (w f) -> p f w", f=D),
                    axis=mybir.AxisListType.X, op=OP.add)
                nc.vector.tensor_reduce(out=packed[:, D:D + 1], in_=exm[:],
                                        axis=mybir.AxisListType.X, op=OP.add)
                pkb = nsb.tile([128, D + 1], bf16, tag="pkb")
                nc.vector.tensor_copy(out=pkb[:], in_=packed[:])
                arp = ps.tile([D + 1, 1], f32, tag="small")
                nc.tensor.matmul(arp[:], lhsT=pkb[:], rhs=ones_c128[:],
                                 start=True, stop=True)
                ar_sb = nsb.tile([D + 1, 1], f32, tag="ar_sb")
                nc.vector.tensor_copy(out=ar_sb[:], in_=arp[:])
                nc.sync.dma_start(ar_ins[s][:], ar_sb[:])
                nc.gpsimd.collective_compute(
                    "AllReduce", OP.add, replica_groups=RG,
                    ins=[ar_ins[s][:].opt()], outs=[ar_outs[s][:].opt()])
                rvsum = nsb.tile([D, 1], f32, tag="rvsum")
                nc.sync.dma_start(rvsum[:], ar_outs[s][:D, :])
                essum = nsb.tile([1, 1], f32, tag="essum")
                nc.sync.dma_start(essum[:], ar_outs[s][D:D + 1, :])
                rec = nsb.tile([1, 1], f32, tag="rec")
                nc.vector.reciprocal(out=rec[:], in_=essum[:])
                recb = nsb.tile([1, 1], bf16, tag="recb")
                nc.vector.tensor_copy(out=recb[:], in_=rec[:])
                rcp = ps.tile([D, 1], f32, tag="small")
                nc.tensor.matmul(rcp[:], lhsT=ones_r128[:, :D], rhs=recb[:],
                                 start=True, stop=True)
                rcs = nsb.tile([D, 1], f32, tag="rcs")
                nc.vector.tensor_copy(out=rcs[:], in_=rcp[:])
                rvs = nsb.tile([D, 1], f32, tag="rvs")
                nc.vector.tensor_tensor(out=rvs[:], in0=rvsum[:], in1=rcs[:],
                                        op=OP.mult)
                nc.vector.tensor_copy(out=qs1[:], in_=hl[:])
                nc.vector.tensor_copy(out=qs2[:], in_=rvs[:])

            # ---- final MLP ---------------------------------------------
            g4 = pp.tile([128, 32 * D], bf16)
            for u in range(32):
                nc.gpsimd.indirect_dma_start(
                    out=g4[:, u * D:(u + 1) * D], out_offset=None,
                    in_=tables[ITERS][:],
                    in_offset=bass.IndirectOffsetOnAxis(
                        ap=g4idx_s[:, u:u + 1], axis=0))

            def outer(qcol, tag):
                qp = ps.tile([1, D], bf16, tag="small")
                nc.tensor.transpose(qp[:], qcol[:], identb[:D, :D])
                qr = nsb.tile([1, D], bf16, tag=f"{tag}r")
                nc.vector.tensor_copy(out=qr[:], in_=qp[:])
                op_ = ps.tile([D, D], f32, tag="small")
                nc.tensor.matmul(op_[:], lhsT=qr[:], rhs=sbar_row[:],
                                 start=True, stop=True)
                ob = nsb.tile([D, D], bf16, tag=f"{tag}b")
                nc.vector.tensor_copy(out=ob[:], in_=op_[:])
                return ob

            oA = outer(qs1, "oA")
            oB = outer(qs2, "oB")
            m1T = pp.tile([D, TCORE], bf16)
            for j in range(2):
                sl = slice(j * 512, (j + 1) * 512)
                yp = ps.tile([D, 512], f32, tag="med")
                nc.tensor.matmul(yp[:], lhsT=wb["lin1_wA"][:],
                                 rhs=g4[:, sl], start=True, stop=False)
                nc.tensor.matmul(yp[:], lhsT=oA[:], rhs=selA_s[:, sl],
                                 start=False, stop=False)
                nc.tensor.matmul(yp[:], lhsT=oB[:], rhs=selB_s[:, sl],
                                 start=False, stop=True)
                nc.scalar.activation(m1T[:, sl], yp[:], AF.Relu,
                                     bias=col("lin1_b"))
            y2 = pp.tile([6, TCORE], f32)
            for j in range(2):
                sl = slice(j * 512, (j + 1) * 512)
                y2p = ps.tile([6, 512], f32, tag="med")
                nc.tensor.matmul(y2p[:], lhsT=wb["lin2_w"][:], rhs=m1T[:, sl],
                                 start=True, stop=True)
                nc.scalar.activation(y2[:, sl], y2p[:], AF.Identity,
                                     bias=col("lin2_b", 6))
            ysb = pp.tile([128, 8 * 6], f32)
            for k in range(8):
                ytp = ps.tile([128, 6], f32, tag="small")
                nc.tensor.transpose(ytp[:], y2[:, k * 128:(k + 1) * 128],
                                    ident[:6, :6])
                nc.vector.tensor_copy(out=ysb[:, k * 6:(k + 1) * 6],
                                      in_=ytp[:])
            nc.sync.dma_start(
                out_d.ap().rearrange("(k p) a -> p k a", p=128),
                ysb[:].rearrange("p (k a) -> p k a", a=6))

    nc.compile()
    return nc


def get_compiled(inputs):
    import hashlib
    h = hashlib.sha1()
    for k in sorted(inputs):
        a = np.ascontiguousarray(np.asarray(inputs[k]))
        h.update(k.encode())
        h.update(a.tobytes()[:65536])
        h.update(str(a.shape).encode())
    key = h.hexdigest()
    if key not in _cache:
        in_maps, weights, colnames, TPW, TILES = _host_prep(inputs)
        nc = _build_graph(weights, colnames, TPW, TILES)
        _cache.clear()
        _cache[key] = (nc, in_maps)
    return _cache[key]


def kernel(**inputs) -> np.ndarray:
    from concourse import bass_utils
    nc, in_maps = get_compiled(inputs)
    res = bass_utils.run_bass_kernel_spmd(nc, in_maps,
                                          core_ids=list(range(C)))
    outs = [np.asarray(r["out"], np.float32) for r in res.results]
    return np.concatenate(outs, 0)
